# revision 27
# baseline (speedup 1.0000x reference)
"""Trainium2 Bass kernel for nn_Att_cov (edge attention + GCN scalar + segment softmax).

Strategy (8 NeuronCores, SPMD, no collectives):
- Node space padded to 51200; core c owns nodes [6400c, 6400(c+1)) = 64 graphs.
  Each core's inputs are ROTATED by 6400c so its own range is [0, 6400) —
  all per-core addressing becomes static (same SPMD program).
- Edges are sharded by destination (col) ownership.  Host sorts each core's
  edges by col into 128 chunks of 50 consecutive nodes (<=960 slots each) and
  by row into 128 chunks of 400-node windows.
- Per-node tables (p1, p2, h=x@Wgcn, dinv, dinv*h, dinv^2*h) are computed on
  device from x via PE matmuls.
- Destination-side per-edge values (p2[col], dinv[col]) are produced by a
  local_scatter of window values to run starts + a segmented carry scan.
- Source-side per-edge values (p1[row], dinv[row]*h[row]) are expanded the
  same way in row-sorted layout, then routed to the col-sorted layout by a
  3-stage permutation network: local_scatter (within partition) -> PE
  transpose -> local_scatter -> PE transpose -> local_scatter.  The routing
  (a proper 960-edge-coloring of the 128x128 slot multigraph) is computed on
  the host by Euler splitting + Koenig matchings.
- GCN sums per node come from a segmented cumulative scan + local_scatter
  extraction of run-end values; per-graph softmax is dense ([16,4,100] tiles).
Host work is limited to integer metadata (sorting/packing/routing of indices)
plus the final unpermutation of device outputs; all floating-point math
happens on the NeuronCores.
"""

import sys

sys.path.insert(0, "/opt/trn_rl_repo")

import numpy as np
import ml_dtypes
import scipy.sparse as sp
from scipy.sparse.csgraph import maximum_bipartite_matching

import concourse.bass as bass
import concourse.tile as tile
from concourse import bacc, mybir, library_config
from concourse.bass_utils import run_bass_kernel_spmd
from concourse.masks import make_identity

BF = ml_dtypes.bfloat16
F32 = np.float32

N_NODES = 50000
N_EDGES = 800000
D = 64
N_GRAPHS = 500
NCORES = 8
NP_ = 51200          # padded node space (1024 chunks * 50)
PCN = NP_ // NCORES  # 6400 nodes per core
CW = 50              # col chunk width (nodes)
RW = 400             # row window width (nodes)
P = 128
F = 960              # slots per chunk
TF = 2 * F           # 1920 (two bf16 halves)
NT = TF // 128       # 15 transpose tiles


# ---------------------------------------------------------------------------
# Host: routing / metadata
# ---------------------------------------------------------------------------

def _euler_split(uu, vv):
    """2-color edges of an even-regular bipartite multigraph so each color
    class is half-regular.  uu/vv: endpoint arrays.  Returns int8 colors."""
    m = len(uu)
    oa = np.argsort(uu, kind="stable")
    pa = np.empty(m, np.int64)
    pa[oa[0::2]] = oa[1::2]
    pa[oa[1::2]] = oa[0::2]
    ob = np.argsort(vv, kind="stable")
    pb = np.empty(m, np.int64)
    pb[ob[0::2]] = ob[1::2]
    pb[ob[1::2]] = ob[0::2]
    color = np.full(m, -1, np.int8)
    pa_l = pa.tolist()
    pb_l = pb.tolist()
    col_l = color.tolist()
    for s in range(m):
        if col_l[s] >= 0:
            continue
        e = s
        c = 0
        use_a = True
        while col_l[e] < 0:
            col_l[e] = c
            e = pa_l[e] if use_a else pb_l[e]
            use_a = not use_a
            c ^= 1
    return np.asarray(col_l, np.int8)


def _color_regular(u, v, deg):
    """Proper edge coloring (deg colors) of a deg-regular bipartite multigraph
    on 128+128 vertices.  Returns color per edge."""
    m = len(u)
    out = np.full(m, -1, np.int32)

    def rec(eids, d, base):
        if d % 2 == 1:
            local = eids.copy()
            for k in range(d):
                uu = u[local]
                vv = v[local]
                A = sp.csr_matrix(
                    (np.ones(len(local), np.int8), (uu, vv)), shape=(P, P)
                )
                mcol = maximum_bipartite_matching(A, perm_type="column")
                assert (mcol >= 0).all(), "no perfect matching (bug)"
                key = uu.astype(np.int64) * P + vv
                order = np.argsort(key, kind="stable")
                skey = key[order]
                want = np.arange(P, dtype=np.int64) * P + mcol
                pos = np.searchsorted(skey, want)
                sel = order[pos]          # one edge id (into local) per row
                out[local[sel]] = base + k
                mask = np.ones(len(local), bool)
                mask[sel] = False
                local = local[mask]
            assert len(local) == 0
            return
        cl = _euler_split(u[eids], v[eids])
        rec(eids[cl == 0], d // 2, base)
        rec(eids[cl == 1], d // 2, base + d // 2)

    rec(np.arange(m, dtype=np.int64), deg, 0)
    assert (out >= 0).all()
    return out


def _run_meta(run_vals, run_starts_pos, width, base):
    """helper: scatter idx array for start values."""
    idx = np.full((P, 2 * width), -1, np.int16)
    # run_vals: local node id per run [list per partition]
    return idx


def preprocess(edge_index):
    """Build all per-core metadata.  Returns dict."""
    row = np.asarray(edge_index[0], np.int64)
    col = np.asarray(edge_index[1], np.int64)
    cnt_global = np.bincount(col, minlength=NP_).astype(np.int32)

    cores = col // PCN
    meta = []
    for c in range(NCORES):
        sel = np.nonzero(cores == c)[0]
        ec = len(sel)
        rcol = (col[sel] - PCN * c).astype(np.int64)            # in [0, 6400)
        rrow = (row[sel] - PCN * c) % NP_                        # rotated rows

        # --- col-sorted layout (destination) ---
        order_c = np.argsort(rcol, kind="stable")
        e_c = sel[order_c]
        rc = rcol[order_c]
        rr_of_c = rrow[order_c]
        q = rc // CW
        ccnt = np.bincount(q, minlength=P)
        assert ccnt.max() <= F, f"col chunk overflow {ccnt.max()}"
        off_c = np.zeros(P + 1, np.int64)
        np.cumsum(ccnt, out=off_c[1:])
        j = np.arange(ec) - off_c[q]
        # run starts: first slot or col change
        newrun_c = np.ones(ec, bool)
        newrun_c[1:] = (rc[1:] != rc[:-1])
        m01c = np.ones((P, F), F32)
        m01c[q[newrun_c], j[newrun_c]] = 0.0
        scol_idx = np.full((P, 2 * CW), -1, np.int16)
        lvals = (rc[newrun_c] - q[newrun_c] * CW).astype(np.int64)
        scol_idx[q[newrun_c], 2 * lvals] = j[newrun_c]
        scol_idx[q[newrun_c], 2 * lvals + 1] = F + j[newrun_c]
        # run ends
        endrun = np.ones(ec, bool)
        endrun[:-1] = (rc[1:] != rc[:-1])
        send_idx = np.full((P, F), -1, np.int16)
        send_idx[q[endrun], j[endrun]] = (rc[endrun] - q[endrun] * CW).astype(np.int16)

        # --- row-sorted layout (source) ---
        order_r = np.argsort(rrow, kind="stable")
        e_r = sel[order_r]
        rw = rrow[order_r]
        pr = rw // RW
        rcnt = np.bincount(pr, minlength=P)
        assert rcnt.max() <= F, f"row chunk overflow {rcnt.max()}"
        off_r = np.zeros(P + 1, np.int64)
        np.cumsum(rcnt, out=off_r[1:])
        i = np.arange(ec) - off_r[pr]
        newrun_r = np.ones(ec, bool)
        newrun_r[1:] = (rw[1:] != rw[:-1])
        m01r = np.ones((P, F), F32)
        m01r[pr[newrun_r], i[newrun_r]] = 0.0
        srow_idx = np.full((P, RW), -1, np.int16)
        lr = (rw[newrun_r] - pr[newrun_r] * RW).astype(np.int64)
        srow_idx[pr[newrun_r], lr] = i[newrun_r]

        # --- bijection src slot -> dst slot over all P*F slots ---
        # per edge: src (pr, i) via order_r ; dst (q, j) via order_c
        src_flat = np.full(P * F, -1, np.int64)   # src slot -> dst slot
        # edge id positions: build edge->slot maps
        dst_slot_of_edge = np.empty(ec, np.int64)
        dst_slot_of_edge[:] = q * F + j
        src_slot_of_edge = pr * F + i
        # map via original edge ids: e_c[t] has dst slot q*F+j at t ; e_r[t2] has src slot
        inv_c = np.empty(ec, np.int64)
        inv_c[np.argsort(e_c, kind="stable")] = np.sort(e_c, kind="stable") * 0  # placeholder
        # simpler: edge id -> dst slot dict via arrays
        tmp_dst = np.empty(ec, np.int64)
        o2 = np.argsort(e_c, kind="stable")
        o3 = np.argsort(e_r, kind="stable")
        # e_c[o2] == e_r[o3] == sorted(sel)
        src_flat_edges_src = src_slot_of_edge[o3]
        src_flat_edges_dst = dst_slot_of_edge[o2]
        src_flat[src_flat_edges_src] = src_flat_edges_dst
        # pads
        pad_src = np.nonzero(src_flat < 0)[0]
        used_dst = np.zeros(P * F, bool)
        used_dst[src_flat_edges_dst] = True
        pad_dst = np.nonzero(~used_dst)[0]
        assert len(pad_src) == len(pad_dst)
        src_flat[pad_src] = pad_dst

        # --- coloring ---
        su = src_flat // F * 0 + np.repeat(np.arange(P), F)   # src row per src slot
        sv = src_flat // F                                     # dst row
        colors = _color_regular(su, sv, F)

        # R1: element (p, pos) -> column color ; halves
        r1 = np.empty((P, TF), np.int16)
        cgrid = colors.reshape(P, F)
        r1[:, :F] = cgrid
        r1[:, F:] = cgrid + F
        # colmatch[color, srcrow] = dstrow ; rowfinal[color, dstrow] = dst j
        colmatch = np.empty((F, P), np.int16)
        colmatch[colors, su] = sv
        rowfinal = np.empty((F, P), np.int16)
        rowfinal[colors, sv] = (src_flat % F).astype(np.int16)
        # R2: position (p', 128t + jsrc) holds element of column k=128t+p'
        # from src row jsrc -> goes to 128t + colmatch[kk, jsrc]
        r2 = np.empty((P, TF), np.int16)
        tt_idx = np.arange(TF)
        tind = tt_idx // 128
        jind = tt_idx % 128
        for pp in range(P):
            k = tind * 128 + pp
            kk = np.where(k < F, k, k - F)
            r2[pp, :] = (tind * 128 + colmatch[kk, jind]).astype(np.int16)
        # R3: row d, column k -> final position
        r3 = np.empty((P, TF), np.int16)
        karr = np.arange(TF)
        kk = np.where(karr < F, karr, karr - F)
        addF = np.where(karr < F, 0, F).astype(np.int16)
        for dd in range(P):
            r3[dd, :] = rowfinal[kk, dd] + addF

        meta.append(
            dict(
                e_c=e_c, q=q, j=j,
                m01c=m01c, scol_idx=scol_idx, send_idx=send_idx,
                m01r=m01r, srow_idx=srow_idx,
                r1=r1, r2=r2, r3=r3,
            )
        )
    return dict(cnt=cnt_global, meta=meta)


# ---------------------------------------------------------------------------
# Device kernel builder
# ---------------------------------------------------------------------------

def build_kernel(b_edge, b_gcn):
    nc = bacc.Bacc("TRN2", target_bir_lowering=False, debug=False,
                   num_devices=NCORES)
    HALF_C = NP_ // 2
    bf = mybir.dt.bfloat16
    f32 = mybir.dt.float32
    i16 = mybir.dt.int16
    i32 = mybir.dt.int32

    xT = nc.declare_dram_parameter("xT", [P, HALF_C], bf, isOutput=False)
    cnt = nc.declare_dram_parameter("cnt", [64, 800], i32, isOutput=False)
    w6 = nc.declare_dram_parameter("w6", [P, 8], bf, isOutput=False)
    # bulk i16: [srow 400 | scol 50 | r1 960 | r2a 896 | r2b 1024 | r3a 960
    #            | r3b 960 | send 960 | m01r.bits 960 | m01c.bits 960]
    IB = RW + CW + F + 896 + 1024 + TF + F + 2 * F
    ibulk = nc.declare_dram_parameter("ibulk", [P, IB], i16, isOutput=False)

    em_o = nc.declare_dram_parameter("em", [P, F], f32, isOutput=True)
    es_o = nc.declare_dram_parameter("es", [P, F], f32, isOutput=True)
    natm_o = nc.declare_dram_parameter("natm", [16, 400], f32, isOutput=True)
    nats_o = nc.declare_dram_parameter("nats", [16, 400], f32, isOutput=True)

    pdram = nc.dram_tensor("pdram", [22 * 70 * 400], bf)

    AF = mybir.ActivationFunctionType
    OP = mybir.AluOpType
    HALF = NP_ // 2  # 25600

    with tile.TileContext(nc) as tc:
        with (
            tc.tile_pool(name="main", bufs=1) as pool,
            tc.tile_pool(name="loop", bufs=8) as lpool,
            tc.tile_pool(name="psA", bufs=6, space="PSUM") as psA,
            tc.tile_pool(name="psT", bufs=2, space="PSUM") as psT,
        ):
            nc.gpsimd.load_library(library_config.local_scatter)

            cnt_t = pool.tile([64, 800], i32)
            nc.sync.dma_start(out=cnt_t[:], in_=cnt[:, :])
            cntf = pool.tile([64, 800], f32)
            nc.vector.tensor_copy(out=cntf[:], in_=cnt_t[:])
            lg = pool.tile([64, 800], f32)
            nc.scalar.activation(out=lg[:], in_=cntf[:], func=AF.Ln, bias=1.0)
            dinv = pool.tile([64, 800], f32)
            nc.scalar.activation(out=dinv[:], in_=lg[:], func=AF.Exp, scale=-0.5)
            # ---- S1: P^T = W6^T @ xT ; 64 matmuls of N=400
            w6_t = pool.tile([P, 8], bf)
            nc.sync.dma_start(out=w6_t[:], in_=w6[:, :])
            # xT resident: [128, HALF] bf16 (A-half on parts 0-63, B on 64-127)
            xts = pool.tile([P, HALF], bf)
            _xts_last = None
            for ch in range(8):
                _xts_last = nc.sync.dma_start(
                    out=xts[:, ch * 3200:(ch + 1) * 3200],
                    in_=xT[:, ch * 3200:(ch + 1) * 3200])
            NG3 = 22  # 64 iterations in 22 groups of 3 (last has 1)
            for g in range(NG3):
                gsz = 3 if g < 21 else 1
                pg = psA.tile([70, 400], f32, space="PSUM", tag="pg")
                for b in range(gsz):
                    it = 3 * g + b
                    nc.tensor.matmul(out=pg[32 * b:32 * b + 6, :],
                                     lhsT=w6_t[:, 0:6],
                                     rhs=xts[:, it * 400:(it + 1) * 400],
                                     start=True, stop=True)
                pc = lpool.tile([70, 400], bf, tag="pc")
                nc.vector.tensor_copy(out=pc[0:32 * (gsz - 1) + 6, :],
                                      in_=pg[0:32 * (gsz - 1) + 6, :])
                nc.gpsimd.dma_start(
                    out=pdram[g * 28000:(g + 1) * 28000].rearrange(
                        "(p n) -> p n", p=70),
                    in_=pc[:])

            # ---- bulk metadata loads
            ib_t = pool.tile([P, RW + CW + F + 896 + 1024 + TF + F + 2 * F], i16)
            _ib_dma = nc.sync.dma_start(out=ib_t[:], in_=ibulk[:, :])
            tile.add_dep_helper(_ib_dma.ins, _xts_last.ins, sync=True,
                                reason="delay meta load behind xT stream")
            srow_t = ib_t[:, 0:RW]
            scol_t = ib_t[:, RW:RW + CW]
            _o = RW + CW
            r1_t = ib_t[:, _o:_o + F]
            r2a_t = ib_t[:, _o + F:_o + F + 896]
            r2b_t = ib_t[:, _o + F + 896:_o + F + 1920]
            r3a_t = ib_t[:, _o + F + 1920:_o + F + 2880]
            r3b_t = ib_t[:, _o + F + 2880:_o + F + 3840]
            send_t = ib_t[:, _o + F + 3840:_o + F + 4800]
            m01r_t = ib_t[:, _o + F + 4800:_o + F + 5760].bitcast(bf)
            m01c_t = ib_t[:, _o + F + 5760:_o + F + 6720].bitcast(bf)

            pdram_v = pdram[:].rearrange("(g n) -> g n", n=28000)
            # read all 6 comps per iter: allp[64, 2400]; iter = 3*g + b
            allp = pool.tile([64, 2400], bf)
            for b in range(3):
                cntg = 22 if b == 0 else 21
                nc.gpsimd.dma_start(
                    out=allp[b:64:3, :],
                    in_=pdram_v[0:cntg, 32 * b * 400:32 * b * 400 + 2400])
            v3 = allp[:].rearrange("p (c n) -> p c n", n=400)
            p1v = v3[:, 0::3, :]   # [64, 2, 400] (A, B)
            p2v = v3[:, 1::3, :]
            hv = v3[:, 2::3, :]

            dinv3 = dinv[:].rearrange("p (c n) -> p c n", n=400)
            dinvh = pool.tile([64, 800], f32)
            dinvh3 = dinvh[:].rearrange("p (c n) -> p c n", n=400)
            nc.vector.tensor_tensor(out=dinvh3, in0=dinv3, in1=hv, op=OP.mult)

            trow_p1 = pool.tile([P, RW], bf)
            nc.vector.tensor_copy(out=trow_p1[0:64, :], in_=p1v[:, 0, :])
            trow_dh = pool.tile([P, RW], bf)
            nc.vector.tensor_copy(out=trow_dh[0:64, :], in_=dinvh[:, 0:400])
            trowB = pool.tile([64, 2 * RW], bf)
            nc.vector.tensor_copy(out=trowB[:, 0:400], in_=p1v[:, 1, :])
            nc.vector.tensor_copy(out=trowB[:, 400:800], in_=dinvh[:, 400:800])
            nc.scalar.dma_start(out=trow_p1[64:128, :], in_=trowB[:, 0:400])
            nc.scalar.dma_start(out=trow_dh[64:128, :], in_=trowB[:, 400:800])

            self_ = pool.tile([64, 800], f32)
            nc.vector.tensor_mul(out=self_[:], in0=dinvh[:], in1=dinv[:])
            tcol_sb = pool.tile([16, 800], bf)
            tcol3 = tcol_sb[:].rearrange("p (n t) -> p n t", t=2)
            nc.vector.tensor_copy(out=tcol3[:, :, 0], in_=p2v[0:16, 0, :])
            tnode_sb = pool.tile([16, 800], f32)
            tnode3 = tnode_sb[:].rearrange("p (n t) -> p n t", t=2)
            nc.vector.tensor_copy(out=tnode3[:, :, 0], in_=dinv[0:16, 0:400])
            nc.vector.tensor_copy(out=tnode3[:, :, 1], in_=self_[0:16, 0:400])

            # ---- bulk metadata loads
            ib_t = pool.tile([P, RW + CW + F + 896 + 1024 + TF + F + 2 * F], i16)
            _ib_dma = nc.sync.dma_start(out=ib_t[:], in_=ibulk[:, :])
            tile.add_dep_helper(_ib_dma.ins, _xts_last.ins, sync=True,
                                reason="delay meta load behind xT stream")
            srow_t = ib_t[:, 0:RW]
            scol_t = ib_t[:, RW:RW + CW]
            _o = RW + CW
            r1_t = ib_t[:, _o:_o + F]
            r2a_t = ib_t[:, _o + F:_o + F + 896]
            r2b_t = ib_t[:, _o + F + 896:_o + F + 1920]
            r3a_t = ib_t[:, _o + F + 1920:_o + F + 2880]
            r3b_t = ib_t[:, _o + F + 2880:_o + F + 3840]
            send_t = ib_t[:, _o + F + 3840:_o + F + 4800]
            m01r_t = ib_t[:, _o + F + 4800:_o + F + 5760].bitcast(bf)
            m01c_t = ib_t[:, _o + F + 5760:_o + F + 6720].bitcast(bf)

            pdram_v = pdram[:].rearrange("(g n) -> g n", n=28000)
            # read all 6 comps per iter: allp[64, 2400]; iter = 3*g + b
            allp = pool.tile([64, 2400], bf)
            for b in range(3):
                cntg = 22 if b == 0 else 21
                nc.gpsimd.dma_start(
                    out=allp[b:64:3, :],
                    in_=pdram_v[0:cntg, 32 * b * 400:32 * b * 400 + 2400])
            v3 = allp[:].rearrange("p (c n) -> p c n", n=400)
            p1v = v3[:, 0::3, :]   # [64, 2, 400] (A, B)
            p2v = v3[:, 1::3, :]
            hv = v3[:, 2::3, :]

            dinv3 = dinv[:].rearrange("p (c n) -> p c n", n=400)
            dinvh = pool.tile([64, 800], f32)
            dinvh3 = dinvh[:].rearrange("p (c n) -> p c n", n=400)
            nc.vector.tensor_tensor(out=dinvh3, in0=dinv3, in1=hv, op=OP.mult)
            self_ = pool.tile([64, 800], f32)
            nc.vector.tensor_mul(out=self_[:], in0=dinvh[:], in1=dinv[:])

            trow = pool.tile([P, 2 * RW], bf)
            trow3 = trow[:].rearrange("p (n t) -> p n t", t=2)
            nc.vector.tensor_copy(out=trow3[0:64, :, 0], in_=p1v[:, 0, :])
            nc.vector.tensor_copy(out=trow3[0:64, :, 1], in_=dinvh[:, 0:400])
            trowB = pool.tile([64, 2 * RW], bf)
            trowB3 = trowB[:].rearrange("p (n t) -> p n t", t=2)
            nc.vector.tensor_copy(out=trowB3[:, :, 0], in_=p1v[:, 1, :])
            nc.vector.tensor_copy(out=trowB3[:, :, 1], in_=dinvh[:, 400:800])
            nc.scalar.dma_start(out=trow[64:128, :], in_=trowB[:])

            tcol_sb = pool.tile([16, 800], bf)
            tcol3 = tcol_sb[:].rearrange("p (n t) -> p n t", t=2)
            nc.vector.tensor_copy(out=tcol3[:, :, 0], in_=p2v[0:16, 0, :])
            tnode_sb = pool.tile([16, 800], f32)
            tnode3 = tnode_sb[:].rearrange("p (n t) -> p n t", t=2)
            nc.vector.tensor_copy(out=tnode3[:, :, 0], in_=dinv[0:16, 0:400])
            nc.vector.tensor_copy(out=tnode3[:, :, 1], in_=self_[0:16, 0:400])

            # ---- S4: row-side expansion + permutation network
            exp_r = pool.tile([P, TF], bf)
            nc.gpsimd.local_scatter(
                out_ap=exp_r[:, 0:F], data_ap=trow_p1[:], idxs_ap=srow_t,
                channels=P, num_elems=F, num_idxs=RW)
            expB = pool.tile([P, TF], bf)
            nc.vector.tensor_tensor_scan(
                out=expB[:, 0:F], data0=m01r_t, data1=exp_r[:, 0:F],
                initial=0.0, op0=OP.mult, op1=OP.add)
            nc.gpsimd.local_scatter(
                out_ap=exp_r[:, F:TF], data_ap=trow_dh[:], idxs_ap=srow_t,
                channels=P, num_elems=F, num_idxs=RW)
            nc.vector.tensor_tensor_scan(
                out=expB[:, F:TF], data0=m01r_t, data1=exp_r[:, F:TF],
                initial=0.0, op0=OP.mult, op1=OP.add)

            ident = pool.tile([P, P], bf)
            make_identity(nc, ident[:])

            r1out = pool.tile([P, TF], bf)
            nc.gpsimd.local_scatter(
                out_ap=r1out[:, 0:F], data_ap=expB[:, 0:F], idxs_ap=r1_t,
                channels=P, num_elems=F, num_idxs=F)
            nc.gpsimd.local_scatter(
                out_ap=r1out[:, F:TF], data_ap=expB[:, F:TF], idxs_ap=r1_t,
                channels=P, num_elems=F, num_idxs=F)
            def transpose_grp(dst, srctile, t0, t1):
                pt = psT.tile([P, 8 * 128], bf, space="PSUM", tag="pt")
                for t in range(t0, t1):
                    nc.tensor.transpose(
                        out=pt[:, (t - t0) * 128:(t - t0 + 1) * 128],
                        in_=srctile[:, t * 128:(t + 1) * 128],
                        identity=ident[:])
                nc.vector.tensor_copy(
                    out=dst[:, t0 * 128:t1 * 128],
                    in_=pt[:, 0:(t1 - t0) * 128])

            tt = pool.tile([P, TF], bf)
            transpose_grp(tt, r1out, 0, 7)
            transpose_grp(tt, r1out, 7, 15)
            r2out = pool.tile([P, TF], bf)
            nc.gpsimd.local_scatter(
                out_ap=r2out[:, 0:896], data_ap=tt[:, 0:896], idxs_ap=r2a_t,
                channels=P, num_elems=896, num_idxs=896)
            nc.gpsimd.local_scatter(
                out_ap=r2out[:, 896:TF], data_ap=tt[:, 896:TF], idxs_ap=r2b_t,
                channels=P, num_elems=1024, num_idxs=1024)
            # ---- S3: col-side expansion (p2 per dst slot)
            win_c = pool.tile([P, CW], bf)
            nc.sync.dma_start(out=win_c[:], in_=tcol3[:, :, 0])
            expC = pool.tile([P, F], bf)
            nc.gpsimd.local_scatter(
                out_ap=expC[:], data_ap=win_c[:], idxs_ap=scol_t,
                channels=P, num_elems=F, num_idxs=CW)
            p2e = pool.tile([P, F], f32)
            nc.vector.tensor_tensor_scan(
                out=p2e[:], data0=m01c_t, data1=expC[:], initial=0.0,
                op0=OP.mult, op1=OP.add)

            mid = pool.tile([P, TF], bf)
            transpose_grp(mid, r2out, 0, 7)
            transpose_grp(mid, r2out, 7, 15)
            r3out = pool.tile([P, TF], bf)
            nc.gpsimd.local_scatter(
                out_ap=r3out[:, F:TF], data_ap=mid[:, F:TF], idxs_ap=r3b_t,
                channels=P, num_elems=F, num_idxs=F)

            Ct = pool.tile([P, F], f32)
            nc.vector.tensor_tensor_scan(
                out=Ct[:], data0=m01c_t, data1=r3out[:, F:TF], initial=0.0,
                op0=OP.mult, op1=OP.add)
            C_bf = pool.tile([P, F], bf)
            nc.vector.tensor_copy(out=C_bf[:], in_=Ct[:])

            nc.gpsimd.local_scatter(
                out_ap=r3out[:, 0:F], data_ap=mid[:, 0:F], idxs_ap=r3a_t,
                channels=P, num_elems=F, num_idxs=F)
            S_loc = pool.tile([P, 64], bf)
            nc.gpsimd.local_scatter(
                out_ap=S_loc[:], data_ap=C_bf[:], idxs_ap=send_t,
                channels=P, num_elems=64, num_idxs=F)

            # ---- S5: combine in col space
            esum = pool.tile([P, F], f32)
            nc.vector.scalar_tensor_tensor(
                out=esum[:], in0=r3out[:, 0:F], scalar=float(b_edge),
                op0=OP.add, op1=OP.add, in1=p2e[:])
            em_t = pool.tile([P, F], f32)
            nc.scalar.activation(out=em_t[:], in_=esum[:], func=AF.Sigmoid)
            nc.sync.dma_start(out=em_o[:, :], in_=em_t[:])
            es_t = pool.tile([P, F], f32)
            nc.vector.tensor_scalar(out=es_t[:], in0=em_t[:], scalar1=-1.0,
                                    scalar2=1.0, op0=OP.mult, op1=OP.add)
            nc.sync.dma_start(out=es_o[:, :], in_=es_t[:])
            S_f = pool.tile([P, CW], f32)
            nc.vector.tensor_copy(out=S_f[:], in_=S_loc[:, 0:CW])

            win_n = pool.tile([P, 2 * CW], f32)
            nc.sync.dma_start(out=win_n[:], in_=tnode_sb[:])
            win_n3 = win_n[:].rearrange("p (n t) -> p n t", t=2)
            natt = pool.tile([P, CW], f32)
            nc.vector.tensor_mul(out=natt[:], in0=S_f[:], in1=win_n3[:, :, 0])
            nc.vector.tensor_add(out=natt[:], in0=natt[:], in1=win_n3[:, :, 1])
            nc.vector.tensor_scalar_add(natt[:], natt[:], float(b_gcn))

            # ---- S6: per-graph softmax x2 on [16, 4, 100]
            nf = pool.tile([16, 400], f32)
            nc.sync.dma_start(out=nf[:], in_=natt[:])
            nf3 = nf[:].rearrange("p (g n) -> p g n", n=100)

            def softmax(src3, out_tile, tagp):
                mx = pool.tile([16, 4], f32, tag=f"mx{tagp}")
                nc.vector.tensor_reduce(
                    out=mx[:].rearrange("p (g o) -> p g o", o=1), in_=src3,
                    axis=mybir.AxisListType.X, op=OP.max)
                sub = pool.tile([16, 400], f32, tag=f"sub{tagp}")
                sub3 = sub[:].rearrange("p (g n) -> p g n", n=100)
                nc.vector.tensor_tensor(
                    out=sub3, in0=src3,
                    in1=mx[:].rearrange("p (g o) -> p g o", o=1).to_broadcast([16, 4, 100]),
                    op=OP.subtract)
                ex = pool.tile([16, 400], f32, tag=f"ex{tagp}")
                nc.scalar.activation(out=ex[:], in_=sub[:], func=AF.Exp)
                ex3 = ex[:].rearrange("p (g n) -> p g n", n=100)
                sm = pool.tile([16, 4], f32, tag=f"sm{tagp}")
                nc.vector.tensor_reduce(
                    out=sm[:].rearrange("p (g o) -> p g o", o=1), in_=ex3,
                    axis=mybir.AxisListType.X, op=OP.add)
                rec = pool.tile([16, 4], f32, tag=f"rec{tagp}")
                nc.vector.reciprocal(out=rec[:], in_=sm[:])
                out3 = out_tile[:].rearrange("p (g n) -> p g n", n=100)
                nc.vector.tensor_tensor(
                    out=out3, in0=ex3,
                    in1=rec[:].rearrange("p (g o) -> p g o", o=1).to_broadcast([16, 4, 100]),
                    op=OP.mult)

            natm_t = pool.tile([16, 400], f32)
            softmax(nf3, natm_t, "a")
            nc.sync.dma_start(out=natm_o[:, :], in_=natm_t[:])
            onem = pool.tile([16, 400], f32)
            nc.vector.tensor_scalar(out=onem[:], in0=natm_t[:], scalar1=-1.0,
                                    scalar2=1.0, op0=OP.mult, op1=OP.add)
            onem3 = onem[:].rearrange("p (g n) -> p g n", n=100)
            nats_t = pool.tile([16, 400], f32)
            softmax(onem3, nats_t, "b")
            nc.sync.dma_start(out=nats_o[:, :], in_=nats_t[:])

    nc.compile()
    return nc


# ---------------------------------------------------------------------------
# kernel entry
# ---------------------------------------------------------------------------

def make_in_maps(pre, inputs):
    x = np.asarray(inputs["x"], np.float32)
    W_edge = np.asarray(inputs["W_edge"], np.float32)
    W_gcn = np.asarray(inputs["W_gcn"], np.float32)
    cnt = pre["cnt"]
    xpad = np.zeros((NP_, D), np.float32)
    xpad[:N_NODES] = x
    W3 = np.concatenate([W_edge[:D], W_edge[D:], W_gcn], axis=1)  # [64, 3]
    W6 = np.zeros((P, 8), np.float32)
    W6[0:64, 0:3] = W3
    W6[64:128, 3:6] = W3
    W6 = W6.astype(BF)

    in_maps = []
    for c in range(NCORES):
        rot = np.roll(xpad, -PCN * c, axis=0)          # rotated node space
        xTt = rot.T.astype(BF)
        xT_c = np.ascontiguousarray(
            np.concatenate([xTt[:, :NP_ // 2], xTt[:, NP_ // 2:]], axis=0))
        cr = np.roll(cnt, -PCN * c).reshape(2, 64, 400)
        cnt_c = np.ascontiguousarray(
            np.concatenate([cr[0], cr[1]], axis=1)).astype(np.int32)
        m = pre["meta"][c]
        scol50 = m["scol_idx"][:, 0::2]              # p2 entries only
        r1h = m["r1"][:, :F]                          # cgrid (same both halves)
        r2a = m["r2"][:, :896]
        r2b = (m["r2"][:, 896:] - 896).astype(np.int16)
        r3a = m["r3"][:, :F]
        r3b = (m["r3"][:, F:] - F).astype(np.int16)
        m01r_b = m["m01r"].astype(BF).view(np.int16)
        m01c_b = m["m01c"].astype(BF).view(np.int16)
        ibulk = np.concatenate(
            [m["srow_idx"], scol50, r1h, r2a, r2b, r3a, r3b,
             m["send_idx"], m01r_b, m01c_b], axis=1).astype(np.int16)
        in_maps.append({
            "xT": xT_c, "cnt": cnt_c, "w6": W6, "ibulk": ibulk,
        })
    return in_maps


def kernel(x, edge_index, split_n, W_edge, b_edge, W_gcn, b_gcn):
    edge_index = np.asarray(edge_index)
    b_edge_v = float(np.asarray(b_edge).reshape(-1)[0])
    b_gcn_v = float(np.asarray(b_gcn).reshape(-1)[0])

    pre = preprocess(edge_index)
    in_maps = make_in_maps(pre, dict(x=x, W_edge=W_edge, W_gcn=W_gcn))

    nc = build_kernel(b_edge_v, b_gcn_v)
    res = run_bass_kernel_spmd(nc, in_maps, core_ids=list(range(NCORES)))

    return assemble(pre, res.results)


def assemble(pre, results):
    E = N_EDGES
    edge_m = np.empty(E, np.float32)
    edge_s = np.empty(E, np.float32)
    natm_full = np.empty(NP_, np.float32)
    nats_full = np.empty(NP_, np.float32)
    for c in range(NCORES):
        m = pre["meta"][c]
        em = np.asarray(results[c]["em"], np.float32)
        es = np.asarray(results[c]["es"], np.float32)
        slots = m["q"] * F + m["j"]
        edge_m[m["e_c"]] = em.ravel()[slots]
        edge_s[m["e_c"]] = es.ravel()[slots]
        natm_full[PCN * c: PCN * (c + 1)] = np.asarray(
            results[c]["natm"], np.float32).ravel()
        nats_full[PCN * c: PCN * (c + 1)] = np.asarray(
            results[c]["nats"], np.float32).ravel()
    return (edge_m[:, None], edge_s[:, None],
            natm_full[:N_NODES, None], nats_full[:N_NODES, None])


# revision 28
# speedup vs baseline: 1.1258x; 1.1258x over previous
"""Trainium2 Bass kernel for nn_Att_cov (edge attention + GCN scalar + segment softmax).

Strategy (8 NeuronCores, SPMD, no collectives):
- Node space padded to 51200; core c owns nodes [6400c, 6400(c+1)) = 64 graphs.
  Each core's inputs are ROTATED by 6400c so its own range is [0, 6400) —
  all per-core addressing becomes static (same SPMD program).
- Edges are sharded by destination (col) ownership.  Host sorts each core's
  edges by col into 128 chunks of 50 consecutive nodes (<=960 slots each) and
  by row into 128 chunks of 400-node windows.
- Per-node tables (p1, p2, h=x@Wgcn, dinv, dinv*h, dinv^2*h) are computed on
  device from x via PE matmuls.
- Destination-side per-edge values (p2[col], dinv[col]) are produced by a
  local_scatter of window values to run starts + a segmented carry scan.
- Source-side per-edge values (p1[row], dinv[row]*h[row]) are expanded the
  same way in row-sorted layout, then routed to the col-sorted layout by a
  3-stage permutation network: local_scatter (within partition) -> PE
  transpose -> local_scatter -> PE transpose -> local_scatter.  The routing
  (a proper 960-edge-coloring of the 128x128 slot multigraph) is computed on
  the host by Euler splitting + Koenig matchings.
- GCN sums per node come from a segmented cumulative scan + local_scatter
  extraction of run-end values; per-graph softmax is dense ([16,4,100] tiles).
Host work is limited to integer metadata (sorting/packing/routing of indices)
plus the final unpermutation of device outputs; all floating-point math
happens on the NeuronCores.
"""

import sys

sys.path.insert(0, "/opt/trn_rl_repo")

import numpy as np
import ml_dtypes
import scipy.sparse as sp
from scipy.sparse.csgraph import maximum_bipartite_matching

import concourse.bass as bass
import concourse.tile as tile
from concourse import bacc, mybir, library_config
from concourse.bass_utils import run_bass_kernel_spmd
from concourse.masks import make_identity

BF = ml_dtypes.bfloat16
F32 = np.float32

N_NODES = 50000
N_EDGES = 800000
D = 64
N_GRAPHS = 500
NCORES = 8
NP_ = 51200          # padded node space (1024 chunks * 50)
PCN = NP_ // NCORES  # 6400 nodes per core
CW = 50              # col chunk width (nodes)
RW = 400             # row window width (nodes)
P = 128
F = 960              # slots per chunk
TF = 2 * F           # 1920 (two bf16 halves)
NT = TF // 128       # 15 transpose tiles


# ---------------------------------------------------------------------------
# Host: routing / metadata
# ---------------------------------------------------------------------------

def _euler_split(uu, vv):
    """2-color edges of an even-regular bipartite multigraph so each color
    class is half-regular.  uu/vv: endpoint arrays.  Returns int8 colors."""
    m = len(uu)
    oa = np.argsort(uu, kind="stable")
    pa = np.empty(m, np.int64)
    pa[oa[0::2]] = oa[1::2]
    pa[oa[1::2]] = oa[0::2]
    ob = np.argsort(vv, kind="stable")
    pb = np.empty(m, np.int64)
    pb[ob[0::2]] = ob[1::2]
    pb[ob[1::2]] = ob[0::2]
    color = np.full(m, -1, np.int8)
    pa_l = pa.tolist()
    pb_l = pb.tolist()
    col_l = color.tolist()
    for s in range(m):
        if col_l[s] >= 0:
            continue
        e = s
        c = 0
        use_a = True
        while col_l[e] < 0:
            col_l[e] = c
            e = pa_l[e] if use_a else pb_l[e]
            use_a = not use_a
            c ^= 1
    return np.asarray(col_l, np.int8)


def _color_regular(u, v, deg):
    """Proper edge coloring (deg colors) of a deg-regular bipartite multigraph
    on 128+128 vertices.  Returns color per edge."""
    m = len(u)
    out = np.full(m, -1, np.int32)

    def rec(eids, d, base):
        if d % 2 == 1:
            local = eids.copy()
            for k in range(d):
                uu = u[local]
                vv = v[local]
                A = sp.csr_matrix(
                    (np.ones(len(local), np.int8), (uu, vv)), shape=(P, P)
                )
                mcol = maximum_bipartite_matching(A, perm_type="column")
                assert (mcol >= 0).all(), "no perfect matching (bug)"
                key = uu.astype(np.int64) * P + vv
                order = np.argsort(key, kind="stable")
                skey = key[order]
                want = np.arange(P, dtype=np.int64) * P + mcol
                pos = np.searchsorted(skey, want)
                sel = order[pos]          # one edge id (into local) per row
                out[local[sel]] = base + k
                mask = np.ones(len(local), bool)
                mask[sel] = False
                local = local[mask]
            assert len(local) == 0
            return
        cl = _euler_split(u[eids], v[eids])
        rec(eids[cl == 0], d // 2, base)
        rec(eids[cl == 1], d // 2, base + d // 2)

    rec(np.arange(m, dtype=np.int64), deg, 0)
    assert (out >= 0).all()
    return out


def preprocess(edge_index):
    """Build all per-core metadata.  Returns dict."""
    row = np.asarray(edge_index[0], np.int64)
    col = np.asarray(edge_index[1], np.int64)
    cnt_global = np.bincount(col, minlength=NP_).astype(np.int32)

    cores = col // PCN
    meta = []
    for c in range(NCORES):
        sel = np.nonzero(cores == c)[0]
        ec = len(sel)
        rcol = (col[sel] - PCN * c).astype(np.int64)            # in [0, 6400)
        rrow = (row[sel] - PCN * c) % NP_                        # rotated rows

        # --- col-sorted layout (destination) ---
        order_c = np.argsort(rcol, kind="stable")
        e_c = sel[order_c]
        rc = rcol[order_c]
        rr_of_c = rrow[order_c]
        q = rc // CW
        ccnt = np.bincount(q, minlength=P)
        assert ccnt.max() <= F, f"col chunk overflow {ccnt.max()}"
        off_c = np.zeros(P + 1, np.int64)
        np.cumsum(ccnt, out=off_c[1:])
        j = np.arange(ec) - off_c[q]
        # run starts: first slot or col change
        newrun_c = np.ones(ec, bool)
        newrun_c[1:] = (rc[1:] != rc[:-1])
        m01c = np.ones((P, F), F32)
        m01c[q[newrun_c], j[newrun_c]] = 0.0
        scol_idx = np.full((P, 2 * CW), -1, np.int16)
        lvals = (rc[newrun_c] - q[newrun_c] * CW).astype(np.int64)
        scol_idx[q[newrun_c], 2 * lvals] = j[newrun_c]
        scol_idx[q[newrun_c], 2 * lvals + 1] = F + j[newrun_c]
        # run ends
        endrun = np.ones(ec, bool)
        endrun[:-1] = (rc[1:] != rc[:-1])
        send_idx = np.full((P, F), -1, np.int16)
        send_idx[q[endrun], j[endrun]] = (rc[endrun] - q[endrun] * CW).astype(np.int16)

        # --- row-sorted layout (source) ---
        order_r = np.argsort(rrow, kind="stable")
        e_r = sel[order_r]
        rw = rrow[order_r]
        pr = rw // RW
        rcnt = np.bincount(pr, minlength=P)
        assert rcnt.max() <= F, f"row chunk overflow {rcnt.max()}"
        off_r = np.zeros(P + 1, np.int64)
        np.cumsum(rcnt, out=off_r[1:])
        i = np.arange(ec) - off_r[pr]
        newrun_r = np.ones(ec, bool)
        newrun_r[1:] = (rw[1:] != rw[:-1])
        m01r = np.ones((P, F), F32)
        m01r[pr[newrun_r], i[newrun_r]] = 0.0
        srow_idx = np.full((P, RW), -1, np.int16)
        lr = (rw[newrun_r] - pr[newrun_r] * RW).astype(np.int64)
        srow_idx[pr[newrun_r], lr] = i[newrun_r]

        # --- bijection src slot -> dst slot over all P*F slots ---
        # per edge: src (pr, i) via order_r ; dst (q, j) via order_c
        src_flat = np.full(P * F, -1, np.int64)   # src slot -> dst slot
        # edge id positions: build edge->slot maps
        dst_slot_of_edge = np.empty(ec, np.int64)
        dst_slot_of_edge[:] = q * F + j
        src_slot_of_edge = pr * F + i
        # map via original edge ids: e_c[t] has dst slot q*F+j at t ; e_r[t2] has src slot
        inv_c = np.empty(ec, np.int64)
        inv_c[np.argsort(e_c, kind="stable")] = np.sort(e_c, kind="stable") * 0  # placeholder
        # simpler: edge id -> dst slot dict via arrays
        tmp_dst = np.empty(ec, np.int64)
        o2 = np.argsort(e_c, kind="stable")
        o3 = np.argsort(e_r, kind="stable")
        # e_c[o2] == e_r[o3] == sorted(sel)
        src_flat_edges_src = src_slot_of_edge[o3]
        src_flat_edges_dst = dst_slot_of_edge[o2]
        src_flat[src_flat_edges_src] = src_flat_edges_dst
        # pads
        pad_src = np.nonzero(src_flat < 0)[0]
        used_dst = np.zeros(P * F, bool)
        used_dst[src_flat_edges_dst] = True
        pad_dst = np.nonzero(~used_dst)[0]
        assert len(pad_src) == len(pad_dst)
        src_flat[pad_src] = pad_dst

        # --- coloring ---
        su = src_flat // F * 0 + np.repeat(np.arange(P), F)   # src row per src slot
        sv = src_flat // F                                     # dst row
        colors = _color_regular(su, sv, F)

        # R1: element (p, pos) -> column color ; halves
        r1 = np.empty((P, TF), np.int16)
        cgrid = colors.reshape(P, F)
        r1[:, :F] = cgrid
        r1[:, F:] = cgrid + F
        # colmatch[color, srcrow] = dstrow ; rowfinal[color, dstrow] = dst j
        colmatch = np.empty((F, P), np.int16)
        colmatch[colors, su] = sv
        rowfinal = np.empty((F, P), np.int16)
        rowfinal[colors, sv] = (src_flat % F).astype(np.int16)
        # R2: position (p', 128t + jsrc) holds element of column k=128t+p'
        # from src row jsrc -> goes to 128t + colmatch[kk, jsrc]
        r2 = np.empty((P, TF), np.int16)
        tt_idx = np.arange(TF)
        tind = tt_idx // 128
        jind = tt_idx % 128
        for pp in range(P):
            k = tind * 128 + pp
            kk = np.where(k < F, k, k - F)
            r2[pp, :] = (tind * 128 + colmatch[kk, jind]).astype(np.int16)
        # R3: row d, column k -> final position
        r3 = np.empty((P, TF), np.int16)
        karr = np.arange(TF)
        kk = np.where(karr < F, karr, karr - F)
        addF = np.where(karr < F, 0, F).astype(np.int16)
        for dd in range(P):
            r3[dd, :] = rowfinal[kk, dd] + addF

        meta.append(
            dict(
                e_c=e_c, q=q, j=j,
                m01c=m01c, scol_idx=scol_idx, send_idx=send_idx,
                m01r=m01r, srow_idx=srow_idx,
                r1=r1, r2=r2, r3=r3,
            )
        )
    return dict(cnt=cnt_global, meta=meta)


# ---------------------------------------------------------------------------
# Device kernel builder
# ---------------------------------------------------------------------------

def build_kernel(b_edge, b_gcn):
    nc = bacc.Bacc("TRN2", target_bir_lowering=False, debug=False,
                   num_devices=NCORES)
    HALF_C = NP_ // 2
    bf = mybir.dt.bfloat16
    f32 = mybir.dt.float32
    i16 = mybir.dt.int16
    i32 = mybir.dt.int32

    xT = nc.declare_dram_parameter("xT", [P, HALF_C], bf, isOutput=False)
    cnt = nc.declare_dram_parameter("cnt", [64, 800], i32, isOutput=False)
    w6 = nc.declare_dram_parameter("w6", [P, 8], bf, isOutput=False)
    # bulk i16: [srow 400 | scol 50 | r1 960 | r2a 896 | r2b 1024 | r3a 960
    #            | r3b 960 | send 960 | m01r.bits 960 | m01c.bits 960]
    IB = RW + CW + F + 896 + 1024 + TF + F + 2 * F
    ibulk = nc.declare_dram_parameter("ibulk", [P, IB], i16, isOutput=False)

    em_o = nc.declare_dram_parameter("em", [P, F], f32, isOutput=True)
    es_o = nc.declare_dram_parameter("es", [P, F], f32, isOutput=True)
    natm_o = nc.declare_dram_parameter("natm", [16, 400], f32, isOutput=True)
    nats_o = nc.declare_dram_parameter("nats", [16, 400], f32, isOutput=True)

    pdram = nc.dram_tensor("pdram", [22 * 70 * 400], bf)

    AF = mybir.ActivationFunctionType
    OP = mybir.AluOpType
    HALF = NP_ // 2  # 25600

    with tile.TileContext(nc) as tc:
        with (
            tc.tile_pool(name="main", bufs=1) as pool,
            tc.tile_pool(name="loop", bufs=8) as lpool,
            tc.tile_pool(name="psA", bufs=6, space="PSUM") as psA,
            tc.tile_pool(name="psT", bufs=2, space="PSUM") as psT,
        ):
            nc.gpsimd.load_library(library_config.local_scatter)

            cnt_t = pool.tile([64, 800], i32)
            nc.sync.dma_start(out=cnt_t[:], in_=cnt[:, :])
            cntf = pool.tile([64, 800], f32)
            nc.vector.tensor_copy(out=cntf[:], in_=cnt_t[:])
            lg = pool.tile([64, 800], f32)
            nc.scalar.activation(out=lg[:], in_=cntf[:], func=AF.Ln, bias=1.0)
            dinv = pool.tile([64, 800], f32)
            nc.scalar.activation(out=dinv[:], in_=lg[:], func=AF.Exp, scale=-0.5)
            # ---- S1: P^T = W6^T @ xT ; 64 matmuls of N=400
            w6_t = pool.tile([P, 8], bf)
            nc.sync.dma_start(out=w6_t[:], in_=w6[:, :])
            # xT resident: [128, HALF] bf16 (A-half on parts 0-63, B on 64-127)
            xts = pool.tile([P, HALF], bf)
            _xts_last = None
            for ch in range(8):
                _xts_last = nc.sync.dma_start(
                    out=xts[:, ch * 3200:(ch + 1) * 3200],
                    in_=xT[:, ch * 3200:(ch + 1) * 3200])
            NG3 = 22  # 64 iterations in 22 groups of 3 (last has 1)
            for g in range(NG3):
                gsz = 3 if g < 21 else 1
                pg = psA.tile([70, 400], f32, space="PSUM", tag="pg")
                for b in range(gsz):
                    it = 3 * g + b
                    nc.tensor.matmul(out=pg[32 * b:32 * b + 6, :],
                                     lhsT=w6_t[:, 0:6],
                                     rhs=xts[:, it * 400:(it + 1) * 400],
                                     start=True, stop=True)
                pc = lpool.tile([70, 400], bf, tag="pc")
                nc.vector.tensor_copy(out=pc[0:32 * (gsz - 1) + 6, :],
                                      in_=pg[0:32 * (gsz - 1) + 6, :])
                nc.gpsimd.dma_start(
                    out=pdram[g * 28000:(g + 1) * 28000].rearrange(
                        "(p n) -> p n", p=70),
                    in_=pc[:])

            # ---- bulk metadata loads
            ib_t = pool.tile([P, RW + CW + F + 896 + 1024 + TF + F + 2 * F], i16)
            _ib_dma = nc.sync.dma_start(out=ib_t[:], in_=ibulk[:, :])
            tile.add_dep_helper(_ib_dma.ins, _xts_last.ins, sync=True,
                                reason="delay meta load behind xT stream")
            srow_t = ib_t[:, 0:RW]
            scol_t = ib_t[:, RW:RW + CW]
            _o = RW + CW
            r1_t = ib_t[:, _o:_o + F]
            r2a_t = ib_t[:, _o + F:_o + F + 896]
            r2b_t = ib_t[:, _o + F + 896:_o + F + 1920]
            r3a_t = ib_t[:, _o + F + 1920:_o + F + 2880]
            r3b_t = ib_t[:, _o + F + 2880:_o + F + 3840]
            send_t = ib_t[:, _o + F + 3840:_o + F + 4800]
            m01r_t = ib_t[:, _o + F + 4800:_o + F + 5760].bitcast(bf)
            m01c_t = ib_t[:, _o + F + 5760:_o + F + 6720].bitcast(bf)

            pdram_v = pdram[:].rearrange("(g n) -> g n", n=28000)
            # read all 6 comps per iter: allp[64, 2400]; iter = 3*g + b
            allp = pool.tile([64, 2400], bf)
            for b in range(3):
                cntg = 22 if b == 0 else 21
                nc.gpsimd.dma_start(
                    out=allp[b:64:3, :],
                    in_=pdram_v[0:cntg, 32 * b * 400:32 * b * 400 + 2400])
            v3 = allp[:].rearrange("p (c n) -> p c n", n=400)
            p1v = v3[:, 0::3, :]   # [64, 2, 400] (A, B)
            p2v = v3[:, 1::3, :]
            hv = v3[:, 2::3, :]

            dinv3 = dinv[:].rearrange("p (c n) -> p c n", n=400)
            dinvh = pool.tile([64, 800], f32)
            dinvh3 = dinvh[:].rearrange("p (c n) -> p c n", n=400)
            nc.vector.tensor_tensor(out=dinvh3, in0=dinv3, in1=hv, op=OP.mult)

            trow_p1 = pool.tile([P, RW], bf)
            nc.vector.tensor_copy(out=trow_p1[0:64, :], in_=p1v[:, 0, :])
            trow_dh = pool.tile([P, RW], bf)
            nc.vector.tensor_copy(out=trow_dh[0:64, :], in_=dinvh[:, 0:400])
            trowB = pool.tile([64, 2 * RW], bf)
            nc.vector.tensor_copy(out=trowB[:, 0:400], in_=p1v[:, 1, :])
            nc.vector.tensor_copy(out=trowB[:, 400:800], in_=dinvh[:, 400:800])
            nc.scalar.dma_start(out=trow_p1[64:128, :], in_=trowB[:, 0:400])
            nc.scalar.dma_start(out=trow_dh[64:128, :], in_=trowB[:, 400:800])

            self_ = pool.tile([64, 800], f32)
            nc.vector.tensor_mul(out=self_[:], in0=dinvh[:], in1=dinv[:])
            tcol_sb = pool.tile([16, 800], bf)
            tcol3 = tcol_sb[:].rearrange("p (n t) -> p n t", t=2)
            nc.vector.tensor_copy(out=tcol3[:, :, 0], in_=p2v[0:16, 0, :])
            tnode_sb = pool.tile([16, 800], f32)
            tnode3 = tnode_sb[:].rearrange("p (n t) -> p n t", t=2)
            nc.vector.tensor_copy(out=tnode3[:, :, 0], in_=dinv[0:16, 0:400])
            nc.vector.tensor_copy(out=tnode3[:, :, 1], in_=self_[0:16, 0:400])

            # ---- bulk metadata loads
            ib_t = pool.tile([P, RW + CW + F + 896 + 1024 + TF + F + 2 * F], i16)
            _ib_dma = nc.sync.dma_start(out=ib_t[:], in_=ibulk[:, :])
            tile.add_dep_helper(_ib_dma.ins, _xts_last.ins, sync=True,
                                reason="delay meta load behind xT stream")
            srow_t = ib_t[:, 0:RW]
            scol_t = ib_t[:, RW:RW + CW]
            _o = RW + CW
            r1_t = ib_t[:, _o:_o + F]
            r2a_t = ib_t[:, _o + F:_o + F + 896]
            r2b_t = ib_t[:, _o + F + 896:_o + F + 1920]
            r3a_t = ib_t[:, _o + F + 1920:_o + F + 2880]
            r3b_t = ib_t[:, _o + F + 2880:_o + F + 3840]
            send_t = ib_t[:, _o + F + 3840:_o + F + 4800]
            m01r_t = ib_t[:, _o + F + 4800:_o + F + 5760].bitcast(bf)
            m01c_t = ib_t[:, _o + F + 5760:_o + F + 6720].bitcast(bf)

            pdram_v = pdram[:].rearrange("(g n) -> g n", n=28000)
            # read all 6 comps per iter: allp[64, 2400]; iter = 3*g + b
            allp = pool.tile([64, 2400], bf)
            for b in range(3):
                cntg = 22 if b == 0 else 21
                nc.gpsimd.dma_start(
                    out=allp[b:64:3, :],
                    in_=pdram_v[0:cntg, 32 * b * 400:32 * b * 400 + 2400])
            v3 = allp[:].rearrange("p (c n) -> p c n", n=400)
            p1v = v3[:, 0::3, :]   # [64, 2, 400] (A, B)
            p2v = v3[:, 1::3, :]
            hv = v3[:, 2::3, :]

            dinv3 = dinv[:].rearrange("p (c n) -> p c n", n=400)
            dinvh = pool.tile([64, 800], f32)
            dinvh3 = dinvh[:].rearrange("p (c n) -> p c n", n=400)
            nc.vector.tensor_tensor(out=dinvh3, in0=dinv3, in1=hv, op=OP.mult)
            self_ = pool.tile([64, 800], f32)
            nc.vector.tensor_mul(out=self_[:], in0=dinvh[:], in1=dinv[:])

            trow = pool.tile([P, 2 * RW], bf)
            trow3 = trow[:].rearrange("p (n t) -> p n t", t=2)
            nc.vector.tensor_copy(out=trow3[0:64, :, 0], in_=p1v[:, 0, :])
            nc.vector.tensor_copy(out=trow3[0:64, :, 1], in_=dinvh[:, 0:400])
            trowB = pool.tile([64, 2 * RW], bf)
            trowB3 = trowB[:].rearrange("p (n t) -> p n t", t=2)
            nc.vector.tensor_copy(out=trowB3[:, :, 0], in_=p1v[:, 1, :])
            nc.vector.tensor_copy(out=trowB3[:, :, 1], in_=dinvh[:, 400:800])
            nc.scalar.dma_start(out=trow[64:128, :], in_=trowB[:])

            tcol_sb = pool.tile([16, 800], bf)
            tcol3 = tcol_sb[:].rearrange("p (n t) -> p n t", t=2)
            nc.vector.tensor_copy(out=tcol3[:, :, 0], in_=p2v[0:16, 0, :])
            tnode_sb = pool.tile([16, 800], f32)
            tnode3 = tnode_sb[:].rearrange("p (n t) -> p n t", t=2)
            nc.vector.tensor_copy(out=tnode3[:, :, 0], in_=dinv[0:16, 0:400])
            nc.vector.tensor_copy(out=tnode3[:, :, 1], in_=self_[0:16, 0:400])

            # ---- S4: row-side expansion + permutation network
            exp_r = pool.tile([P, TF], bf)
            nc.gpsimd.local_scatter(
                out_ap=exp_r[:, 0:F], data_ap=trow_p1[:], idxs_ap=srow_t,
                channels=P, num_elems=F, num_idxs=RW)
            expB = pool.tile([P, TF], bf)
            nc.vector.tensor_tensor_scan(
                out=expB[:, 0:F], data0=m01r_t, data1=exp_r[:, 0:F],
                initial=0.0, op0=OP.mult, op1=OP.add)
            nc.gpsimd.local_scatter(
                out_ap=exp_r[:, F:TF], data_ap=trow_dh[:], idxs_ap=srow_t,
                channels=P, num_elems=F, num_idxs=RW)
            nc.vector.tensor_tensor_scan(
                out=expB[:, F:TF], data0=m01r_t, data1=exp_r[:, F:TF],
                initial=0.0, op0=OP.mult, op1=OP.add)

            ident = pool.tile([P, P], bf)
            make_identity(nc, ident[:])

            r1out = pool.tile([P, TF], bf)
            nc.gpsimd.local_scatter(
                out_ap=r1out[:, 0:F], data_ap=expB[:, 0:F], idxs_ap=r1_t,
                channels=P, num_elems=F, num_idxs=F)
            nc.gpsimd.local_scatter(
                out_ap=r1out[:, F:TF], data_ap=expB[:, F:TF], idxs_ap=r1_t,
                channels=P, num_elems=F, num_idxs=F)
            def transpose_grp(dst, srctile, t0, t1):
                pt = psT.tile([P, 8 * 128], bf, space="PSUM", tag="pt")
                for t in range(t0, t1):
                    nc.tensor.transpose(
                        out=pt[:, (t - t0) * 128:(t - t0 + 1) * 128],
                        in_=srctile[:, t * 128:(t + 1) * 128],
                        identity=ident[:])
                nc.vector.tensor_copy(
                    out=dst[:, t0 * 128:t1 * 128],
                    in_=pt[:, 0:(t1 - t0) * 128])

            tt = pool.tile([P, TF], bf)
            transpose_grp(tt, r1out, 0, 7)
            transpose_grp(tt, r1out, 7, 15)
            r2out = pool.tile([P, TF], bf)
            nc.gpsimd.local_scatter(
                out_ap=r2out[:, 0:896], data_ap=tt[:, 0:896], idxs_ap=r2a_t,
                channels=P, num_elems=896, num_idxs=896)
            nc.gpsimd.local_scatter(
                out_ap=r2out[:, 896:TF], data_ap=tt[:, 896:TF], idxs_ap=r2b_t,
                channels=P, num_elems=1024, num_idxs=1024)
            # ---- S3: col-side expansion (p2 per dst slot)
            win_c = pool.tile([P, CW], bf)
            nc.sync.dma_start(out=win_c[:], in_=tcol3[:, :, 0])
            expC = pool.tile([P, F], bf)
            nc.gpsimd.local_scatter(
                out_ap=expC[:], data_ap=win_c[:], idxs_ap=scol_t,
                channels=P, num_elems=F, num_idxs=CW)
            p2e = pool.tile([P, F], f32)
            nc.vector.tensor_tensor_scan(
                out=p2e[:], data0=m01c_t, data1=expC[:], initial=0.0,
                op0=OP.mult, op1=OP.add)

            mid = pool.tile([P, TF], bf)
            transpose_grp(mid, r2out, 0, 7)
            transpose_grp(mid, r2out, 7, 15)
            r3out = pool.tile([P, TF], bf)
            nc.gpsimd.local_scatter(
                out_ap=r3out[:, F:TF], data_ap=mid[:, F:TF], idxs_ap=r3b_t,
                channels=P, num_elems=F, num_idxs=F)

            Ct = pool.tile([P, F], f32)
            nc.vector.tensor_tensor_scan(
                out=Ct[:], data0=m01c_t, data1=r3out[:, F:TF], initial=0.0,
                op0=OP.mult, op1=OP.add)
            C_bf = pool.tile([P, F], bf)
            nc.vector.tensor_copy(out=C_bf[:], in_=Ct[:])

            nc.gpsimd.local_scatter(
                out_ap=r3out[:, 0:F], data_ap=mid[:, 0:F], idxs_ap=r3a_t,
                channels=P, num_elems=F, num_idxs=F)
            S_loc = pool.tile([P, 64], bf)
            nc.gpsimd.local_scatter(
                out_ap=S_loc[:], data_ap=C_bf[:], idxs_ap=send_t,
                channels=P, num_elems=64, num_idxs=F)

            # ---- S5: combine in col space
            esum = pool.tile([P, F], f32)
            nc.vector.scalar_tensor_tensor(
                out=esum[:], in0=r3out[:, 0:F], scalar=float(b_edge),
                op0=OP.add, op1=OP.add, in1=p2e[:])
            em_t = pool.tile([P, F], f32)
            nc.scalar.activation(out=em_t[:], in_=esum[:], func=AF.Sigmoid)
            nc.sync.dma_start(out=em_o[:, :], in_=em_t[:])
            es_t = pool.tile([P, F], f32)
            nc.vector.tensor_scalar(out=es_t[:], in0=em_t[:], scalar1=-1.0,
                                    scalar2=1.0, op0=OP.mult, op1=OP.add)
            nc.sync.dma_start(out=es_o[:, :], in_=es_t[:])
            S_f = pool.tile([P, CW], f32)
            nc.vector.tensor_copy(out=S_f[:], in_=S_loc[:, 0:CW])

            win_n = pool.tile([P, 2 * CW], f32)
            nc.sync.dma_start(out=win_n[:], in_=tnode_sb[:])
            win_n3 = win_n[:].rearrange("p (n t) -> p n t", t=2)
            natt = pool.tile([P, CW], f32)
            nc.vector.tensor_mul(out=natt[:], in0=S_f[:], in1=win_n3[:, :, 0])
            nc.vector.tensor_add(out=natt[:], in0=natt[:], in1=win_n3[:, :, 1])
            nc.vector.tensor_scalar_add(natt[:], natt[:], float(b_gcn))

            # ---- S6: per-graph softmax x2 on [16, 4, 100]
            nf = pool.tile([16, 400], f32)
            nc.sync.dma_start(out=nf[:], in_=natt[:])
            nf3 = nf[:].rearrange("p (g n) -> p g n", n=100)

            def softmax(src3, out_tile, tagp):
                mx = pool.tile([16, 4], f32, tag=f"mx{tagp}")
                nc.vector.tensor_reduce(
                    out=mx[:].rearrange("p (g o) -> p g o", o=1), in_=src3,
                    axis=mybir.AxisListType.X, op=OP.max)
                sub = pool.tile([16, 400], f32, tag=f"sub{tagp}")
                sub3 = sub[:].rearrange("p (g n) -> p g n", n=100)
                nc.vector.tensor_tensor(
                    out=sub3, in0=src3,
                    in1=mx[:].rearrange("p (g o) -> p g o", o=1).to_broadcast([16, 4, 100]),
                    op=OP.subtract)
                ex = pool.tile([16, 400], f32, tag=f"ex{tagp}")
                nc.scalar.activation(out=ex[:], in_=sub[:], func=AF.Exp)
                ex3 = ex[:].rearrange("p (g n) -> p g n", n=100)
                sm = pool.tile([16, 4], f32, tag=f"sm{tagp}")
                nc.vector.tensor_reduce(
                    out=sm[:].rearrange("p (g o) -> p g o", o=1), in_=ex3,
                    axis=mybir.AxisListType.X, op=OP.add)
                rec = pool.tile([16, 4], f32, tag=f"rec{tagp}")
                nc.vector.reciprocal(out=rec[:], in_=sm[:])
                out3 = out_tile[:].rearrange("p (g n) -> p g n", n=100)
                nc.vector.tensor_tensor(
                    out=out3, in0=ex3,
                    in1=rec[:].rearrange("p (g o) -> p g o", o=1).to_broadcast([16, 4, 100]),
                    op=OP.mult)

            natm_t = pool.tile([16, 400], f32)
            softmax(nf3, natm_t, "a")
            nc.sync.dma_start(out=natm_o[:, :], in_=natm_t[:])
            onem = pool.tile([16, 400], f32)
            nc.vector.tensor_scalar(out=onem[:], in0=natm_t[:], scalar1=-1.0,
                                    scalar2=1.0, op0=OP.mult, op1=OP.add)
            onem3 = onem[:].rearrange("p (g n) -> p g n", n=100)
            nats_t = pool.tile([16, 400], f32)
            softmax(onem3, nats_t, "b")
            nc.sync.dma_start(out=nats_o[:, :], in_=nats_t[:])

    nc.compile()
    return nc


# ---------------------------------------------------------------------------
# kernel entry
# ---------------------------------------------------------------------------

def make_in_maps(pre, inputs):
    x = np.asarray(inputs["x"], np.float32)
    W_edge = np.asarray(inputs["W_edge"], np.float32)
    W_gcn = np.asarray(inputs["W_gcn"], np.float32)
    cnt = pre["cnt"]
    xpad = np.zeros((NP_, D), np.float32)
    xpad[:N_NODES] = x
    W3 = np.concatenate([W_edge[:D], W_edge[D:], W_gcn], axis=1)  # [64, 3]
    W6 = np.zeros((P, 8), np.float32)
    W6[0:64, 0:3] = W3
    W6[64:128, 3:6] = W3
    W6 = W6.astype(BF)

    in_maps = []
    for c in range(NCORES):
        rot = np.roll(xpad, -PCN * c, axis=0)          # rotated node space
        xTt = rot.T.astype(BF)
        xT_c = np.ascontiguousarray(
            np.concatenate([xTt[:, :NP_ // 2], xTt[:, NP_ // 2:]], axis=0))
        cr = np.roll(cnt, -PCN * c).reshape(2, 64, 400)
        cnt_c = np.ascontiguousarray(
            np.concatenate([cr[0], cr[1]], axis=1)).astype(np.int32)
        m = pre["meta"][c]
        scol50 = m["scol_idx"][:, 0::2]              # p2 entries only
        r1h = m["r1"][:, :F]                          # cgrid (same both halves)
        r2a = m["r2"][:, :896]
        r2b = (m["r2"][:, 896:] - 896).astype(np.int16)
        r3a = m["r3"][:, :F]
        r3b = (m["r3"][:, F:] - F).astype(np.int16)
        m01r_b = m["m01r"].astype(BF).view(np.int16)
        m01c_b = m["m01c"].astype(BF).view(np.int16)
        ibulk = np.concatenate(
            [m["srow_idx"], scol50, r1h, r2a, r2b, r3a, r3b,
             m["send_idx"], m01r_b, m01c_b], axis=1).astype(np.int16)
        in_maps.append({
            "xT": xT_c, "cnt": cnt_c, "w6": W6, "ibulk": ibulk,
        })
    return in_maps


def kernel(x, edge_index, split_n, W_edge, b_edge, W_gcn, b_gcn):
    edge_index = np.asarray(edge_index)
    b_edge_v = float(np.asarray(b_edge).reshape(-1)[0])
    b_gcn_v = float(np.asarray(b_gcn).reshape(-1)[0])

    pre = preprocess(edge_index)
    in_maps = make_in_maps(pre, dict(x=x, W_edge=W_edge, W_gcn=W_gcn))

    nc = build_kernel(b_edge_v, b_gcn_v)
    res = run_bass_kernel_spmd(nc, in_maps, core_ids=list(range(NCORES)))

    return assemble(pre, res.results)


def assemble(pre, results):
    E = N_EDGES
    edge_m = np.empty(E, np.float32)
    edge_s = np.empty(E, np.float32)
    natm_full = np.empty(NP_, np.float32)
    nats_full = np.empty(NP_, np.float32)
    for c in range(NCORES):
        m = pre["meta"][c]
        em = np.asarray(results[c]["em"], np.float32)
        es = np.asarray(results[c]["es"], np.float32)
        slots = m["q"] * F + m["j"]
        edge_m[m["e_c"]] = em.ravel()[slots]
        edge_s[m["e_c"]] = es.ravel()[slots]
        natm_full[PCN * c: PCN * (c + 1)] = np.asarray(
            results[c]["natm"], np.float32).ravel()
        nats_full[PCN * c: PCN * (c + 1)] = np.asarray(
            results[c]["nats"], np.float32).ravel()
    return (edge_m[:, None], edge_s[:, None],
            natm_full[:N_NODES, None], nats_full[:N_NODES, None])
